# revision 35
# baseline (speedup 1.0000x reference)
"""DIEN (Deep Interest Evolution Network) Trainium2 kernel.

Contract: kernel(**inputs) takes FULL unsharded numpy inputs (as produced by
setup_inputs()) and returns the FULL [1024, 1] float32 output.

Strategy:
  - Data-parallel over batch: 1024 rows -> 8 cores x 128 rows.
  - Host dedups each core's embedding indices and ships a compact per-core
    sub-table (<= 25728 rows); the gather itself runs on-device via
    indirect DMA.
  - Main SPMD kernel computes embeddings/GRU/attention/AUGRU per core and
    outputs [128, 164] = [final_state | target_embed | other_info].
  - A second small SPMD kernel computes the FFN head (Dice needs full-batch
    statistics, so its [1024, 164] input is replicated to every core).
"""

import sys
import numpy as np

for _p in ("/opt/trn_rl_repo", "/root/.axon_site/_ro/trn_rl_repo"):
    if _p not in sys.path:
        sys.path.append(_p)

import concourse.bass as bass
import concourse.mybir as mybir
import concourse.tile as tile
from concourse import bacc
from concourse.bass_utils import run_bass_kernel_spmd
from concourse.masks import make_identity
from contextlib import ExitStack

F32 = mybir.dt.float32
I32 = mybir.dt.int32
AF = mybir.ActivationFunctionType
OP = mybir.AluOpType

P = 128            # batch rows per core
T = 200            # sequence length
D = 64             # embedding/hidden dim
NSUB = P * T + P   # 25728: worst-case unique rows per core
TG = 8             # timesteps per gather group
NCORES = 8
ACHUNK = 512       # attention chunk (columns of (t, b))


# ----------------------------------------------------------------------------
# Main kernel: embeddings + GRU + attention + AUGRU  ->  [128, 164]
# ----------------------------------------------------------------------------

def _emit_gather_group(nc, g, seqidx_t, sub_table, ident, gpool, xpsum_pool, xt_pool):
    """Gather TG timesteps of embeddings and transpose into an xT tile.

    Returns a [65, TG*128] SBUF tile: rows 0:64 = x^T for the TG timesteps
    (col block j holds timestep g*TG+j), row 64 = ones (bias lane).
    """
    g_t = gpool.tile([P, TG, D], F32, tag="gather")
    nc.gpsimd.indirect_dma_start(
        out=g_t[:], out_offset=None, in_=sub_table[:],
        in_offset=bass.IndirectOffsetOnAxis(ap=seqidx_t[:, g * TG:(g + 1) * TG], axis=0),
    )
    xps = xpsum_pool.tile([D, TG * P], F32, tag="xps")
    for j in range(TG):
        nc.tensor.transpose(out=xps[:, j * P:(j + 1) * P], in_=g_t[:, j, :], identity=ident[:])
    xt = xt_pool.tile([D + 1, TG * P], F32, tag="xt")
    nc.vector.tensor_copy(xt[0:D, :], xps[:])
    nc.gpsimd.memset(xt[D:D + 1, :], 1.0)
    return xt


def _emit_gru_step(nc, t, hT_stat, xT_stat, h_prev, U_ext, W_ext, score, nscore,
                   mask_bias, ps_pool, ps_ht_pool, h_pool, zr_pool, tmp_pool,
                   seq_hT_dst, ident, offload_pool):
    """One (AU)GRU step in [batch=128 partitions, hidden=64 free] layout.

    hT_stat: [64,128] stationary view of h_{t-1}^T (SBUF)
    xT_stat: [65,128] stationary view of x_t^T with ones row (SBUF)
    h_prev:  [128,64] h_{t-1}
    U_ext: [64,256] = [U_zr | U_hh | 0] (rhs)
    W_ext: [65,256] = [W_zr;b_zr | 0 | W_hh;b_hh] (rhs)
    score/nscore: [128,1] APs of +/- attention scores (AUGRU) or None (GRU,
      valid-mask folded into the sigmoid bias instead)
    psum [128,256]: cols 0:128 = zr preact, 128:192 = hu_hh, 192:256 = xw_hh
    Math: h' = (h - a*(z.h)) + a*(z.hh)
    Returns h_t tile [128,64].
    """
    ps = ps_pool.tile([P, 4 * D], F32, tag="ps")
    nc.tensor.matmul(ps[:], lhsT=xT_stat, rhs=W_ext[:], start=True, stop=False)
    nc.tensor.matmul(ps[:], lhsT=hT_stat, rhs=U_ext[:], start=False, stop=True)

    zr = zr_pool.tile([P, 2 * D], F32, tag="zr")
    if mask_bias is not None:
        nc.scalar.activation(zr[:], ps[:, 0:2 * D], AF.Sigmoid, bias=mask_bias, scale=1.0)
    else:
        nc.scalar.activation(zr[:], ps[:, 0:2 * D], AF.Sigmoid, bias=0.0, scale=1.0)

    # off-critical-path: w = h - a*(z.h)
    zh = tmp_pool.tile([P, D], F32, tag="zh")
    offload_pool.tensor_tensor(out=zh[:], in0=zr[:, 0:D], in1=h_prev[:], op=OP.mult)
    w = tmp_pool.tile([P, D], F32, tag="w")
    nsc = nscore if nscore is not None else -1.0
    nc.vector.scalar_tensor_tensor(out=w[:], in0=zh[:], scalar=nsc,
                                   in1=h_prev[:], op0=OP.mult, op1=OP.add)

    # critical path: r -> hh -> h'
    t1 = tmp_pool.tile([P, D], F32, tag="t1")
    nc.vector.tensor_tensor(out=t1[:], in0=zr[:, D:2 * D], in1=ps[:, 2 * D:3 * D], op=OP.mult)
    t2 = tmp_pool.tile([P, D], F32, tag="t2")
    nc.vector.tensor_tensor(out=t2[:], in0=t1[:], in1=ps[:, 3 * D:4 * D], op=OP.add)
    hh = tmp_pool.tile([P, D], F32, tag="hh")
    nc.scalar.activation(hh[:], t2[:], AF.Tanh, bias=0.0, scale=1.0)

    zhh = tmp_pool.tile([P, D], F32, tag="zhh")
    nc.vector.tensor_tensor(out=zhh[:], in0=zr[:, 0:D], in1=hh[:], op=OP.mult)
    h_new = h_pool.tile([P, D], F32, tag="h")
    sc = score if score is not None else 1.0
    nc.vector.scalar_tensor_tensor(out=h_new[:], in0=zhh[:], scalar=sc,
                                   in1=w[:], op0=OP.mult, op1=OP.add)

    # h_t^T for the next step's stationary (and the seq_h arena for the GRU)
    if seq_hT_dst is not None:
        ps_ht = ps_ht_pool.tile([D, P], F32, tag="ps_ht")
        nc.tensor.transpose(out=ps_ht[:], in_=h_new[:], identity=ident[:])
        nc.vector.tensor_copy(seq_hT_dst, ps_ht[:])
    return h_new


def build_main():
    nc = bacc.Bacc("TRN2", target_bir_lowering=False, debug=False)

    sub_table = nc.dram_tensor("sub_table", [NSUB, D], F32, kind="ExternalInput")
    seq_idx = nc.dram_tensor("seq_idx", [P, T], I32, kind="ExternalInput")
    tgt_idx = nc.dram_tensor("tgt_idx", [P, 1], I32, kind="ExternalInput")
    hist = nc.dram_tensor("hist", [P, 1], I32, kind="ExternalInput")
    other = nc.dram_tensor("other", [P, 36], F32, kind="ExternalInput")
    gru_U_d = nc.dram_tensor("gru_U", [D, 4 * D], F32, kind="ExternalInput")
    gru_Wb_d = nc.dram_tensor("gru_Wb", [D + 1, 4 * D], F32, kind="ExternalInput")
    aug_U_d = nc.dram_tensor("aug_U", [D, 4 * D], F32, kind="ExternalInput")
    aug_Wb_d = nc.dram_tensor("aug_Wb", [D + 1, 4 * D], F32, kind="ExternalInput")
    w1q_d = nc.dram_tensor("w1q", [D + 1, 64], F32, kind="ExternalInput")  # [W1q; b1]
    w1k_d = nc.dram_tensor("w1k", [D, 64], F32, kind="ExternalInput")
    w1qk_d = nc.dram_tensor("w1qk", [D, 64], F32, kind="ExternalInput")
    w2_d = nc.dram_tensor("w2", [65, 16], F32, kind="ExternalInput")       # [W2/4; b2]
    w3_d = nc.dram_tensor("w3", [16, 1], F32, kind="ExternalInput")        # W3/4
    out_main = nc.dram_tensor("out_main", [P, 164], F32, kind="ExternalOutput")

    with tile.TileContext(nc) as tc, ExitStack() as top:
        const = top.enter_context(tc.tile_pool(name="const", bufs=1))
        arena_p = top.enter_context(tc.tile_pool(name="arena", bufs=1))

        ident = const.tile([P, P], F32)
        make_identity(nc, ident[:])

        # weights -> SBUF
        gru_U = const.tile([D, 4 * D], F32); nc.sync.dma_start(gru_U[:], gru_U_d[:])
        gru_Wb = const.tile([D + 1, 4 * D], F32); nc.sync.dma_start(gru_Wb[:], gru_Wb_d[:])
        aug_U = const.tile([D, 4 * D], F32); nc.sync.dma_start(aug_U[:], aug_U_d[:])
        aug_Wb = const.tile([D + 1, 4 * D], F32); nc.sync.dma_start(aug_Wb[:], aug_Wb_d[:])
        w1q = const.tile([D + 1, 64], F32); nc.sync.dma_start(w1q[:], w1q_d[:])
        w1k = const.tile([D, 64], F32); nc.sync.dma_start(w1k[:], w1k_d[:])
        w1qk = const.tile([D, 64], F32); nc.sync.dma_start(w1qk[:], w1qk_d[:])
        w2 = const.tile([65, 16], F32); nc.sync.dma_start(w2[:], w2_d[:])
        w3 = const.tile([16, 1], F32); nc.sync.dma_start(w3[:], w3_d[:])

        seqidx_t = const.tile([P, T], I32); nc.sync.dma_start(seqidx_t[:], seq_idx[:])
        tgtidx_t = const.tile([P, 1], I32); nc.sync.dma_start(tgtidx_t[:], tgt_idx[:])
        hist_t = const.tile([P, 1], I32); nc.sync.dma_start(hist_t[:], hist[:])
        other_t = const.tile([P, 36], F32); nc.sync.dma_start(other_t[:], other[:])

        # valid mask helpers: mask_bias = (valid-1)*30 ; mask_neg = (valid-1)*1e9
        iof = const.tile([P, T], F32)
        ioi = const.tile([P, T], I32)
        nc.gpsimd.iota(ioi[:], pattern=[[1, T]], base=0, channel_multiplier=0)
        nc.vector.tensor_copy(iof[:], ioi[:])
        histf = const.tile([P, 1], F32)
        nc.vector.tensor_copy(histf[:], hist_t[:])
        valid = const.tile([P, T], F32)
        nc.vector.tensor_scalar(out=valid[:], in0=iof[:], scalar1=histf[:, 0:1],
                                scalar2=None, op0=OP.is_lt)
        mask_bias = const.tile([P, T], F32)
        nc.vector.tensor_scalar(out=mask_bias[:], in0=valid[:], scalar1=30.0,
                                scalar2=30.0, op0=OP.mult, op1=OP.subtract)
        mask_neg = const.tile([P, T], F32)
        nc.vector.tensor_scalar(out=mask_neg[:], in0=valid[:], scalar1=1e9,
                                scalar2=1e9, op0=OP.mult, op1=OP.subtract)

        # target embedding -> [128,64] raw + [65,128] transposed-with-ones
        tgt_emb = const.tile([P, D], F32)
        nc.gpsimd.indirect_dma_start(
            out=tgt_emb[:], out_offset=None, in_=sub_table[:],
            in_offset=bass.IndirectOffsetOnAxis(ap=tgtidx_t[:, 0:1], axis=0))
        qT = const.tile([D + 1, P], F32)

        # seq_h^T arena: [65, T*128]; row 64 = ones (bias lane for AUGRU x-side)
        seq_hT = arena_p.tile([D + 1, T * P], F32)
        nc.gpsimd.memset(seq_hT[D:D + 1, :], 1.0)

        zeros_hT = const.tile([D, P], F32); nc.gpsimd.memset(zeros_hT[:], 0.0)
        zeros_h = const.tile([P, D], F32); nc.gpsimd.memset(zeros_h[:], 0.0)

        # ---------- GRU phase (pipelined gather + interleaved attention) ------
        # Attention chunk c (TG timesteps) is emitted right after GRU step
        # t = c*TG+TG-1 so its ops fill engine gaps under the serial scan chain.
        scores = const.tile([P, T], F32)
        nscores = const.tile([P, T], F32)
        with ExitStack() as ph:
            gpool = ph.enter_context(tc.tile_pool(name="gather", bufs=4))
            xpsum_pool = ph.enter_context(tc.tile_pool(name="xps", bufs=1, space="PSUM"))
            xt_pool = ph.enter_context(tc.tile_pool(name="xt", bufs=4))
            ps_pool = ph.enter_context(tc.tile_pool(name="ps", bufs=2, space="PSUM"))
            ps_ht_pool = ph.enter_context(tc.tile_pool(name="ps_ht", bufs=1, space="PSUM"))
            h_pool = ph.enter_context(tc.tile_pool(name="h", bufs=4))
            zr_pool = ph.enter_context(tc.tile_pool(name="zr", bufs=2))
            tmp_pool = ph.enter_context(tc.tile_pool(name="tmp", bufs=4))
            a_pool = ph.enter_context(tc.tile_pool(name="ps_att", bufs=1, space="PSUM"))
            lg_pool = ph.enter_context(tc.tile_pool(name="ps_lg", bufs=1, space="PSUM"))
            sb_pool = ph.enter_context(tc.tile_pool(name="att_sb", bufs=2))

            ps_lg = lg_pool.tile([P, T], F32)

            # transpose target embedding while gathers start
            qps = ps_ht_pool.tile([D, P], F32, tag="ps_ht")
            nc.tensor.transpose(out=qps[:], in_=tgt_emb[:], identity=ident[:])
            nc.vector.tensor_copy(qT[0:D, :], qps[:])
            nc.gpsimd.memset(qT[D:D + 1, :], 1.0)

            def emit_att_chunk(c):
                cols = slice(c * TG * P, (c + 1) * TG * P)
                nco = TG * P
                kT = seq_hT[0:D, cols]
                q_rep = qT[:, None, :].to_broadcast([D + 1, TG, P])
                qk = sb_pool.tile([D, nco], F32, tag="qk")
                nc.gpsimd.tensor_tensor(
                    out=qk[:].rearrange("p (t b) -> p t b", t=TG),
                    in0=q_rep[0:D], in1=kT.rearrange("p (t b) -> p t b", t=TG),
                    op=OP.mult)
                ps_a1 = a_pool.tile([D, nco], F32, tag="att")
                hw = nco // 2  # per-matmul width: one PSUM bank (512 fp32)
                for hb in range(2):
                    sl = slice(hb * hw, (hb + 1) * hw)
                    q_rep_h = qT[:, None, :].to_broadcast([D + 1, TG // 2, P])
                    nc.tensor.matmul(ps_a1[:, sl].rearrange("p (t b) -> p t b", t=TG // 2),
                                     lhsT=w1q[:], rhs=q_rep_h, start=True, stop=False)
                    nc.tensor.matmul(ps_a1[:, sl], lhsT=w1k[:], rhs=kT[:, sl],
                                     start=False, stop=False)
                    nc.tensor.matmul(ps_a1[:, sl], lhsT=w1qk[:], rhs=qk[:, sl],
                                     start=False, stop=True)
                relu1 = sb_pool.tile([D, nco], F32, tag="relu1")
                nc.scalar.activation(relu1[:], ps_a1[:], AF.Relu, bias=0.0, scale=1.0)
                p1 = sb_pool.tile([D + 1, nco], F32, tag="p1")  # 4*prelu(a1); ones row
                nc.vector.scalar_tensor_tensor(out=p1[0:D, :], in0=relu1[:], scalar=3.0,
                                               in1=ps_a1[:], op0=OP.mult, op1=OP.add)
                nc.gpsimd.memset(p1[D:D + 1, :], 1.0)
                ps_a2 = a_pool.tile([16, nco], F32, tag="att")
                for hb in range(2):
                    sl = slice(hb * hw, (hb + 1) * hw)
                    nc.tensor.matmul(ps_a2[:, sl], lhsT=w2[:], rhs=p1[:, sl],
                                     start=True, stop=True)
                relu2 = sb_pool.tile([16, nco], F32, tag="relu2")
                nc.scalar.activation(relu2[:], ps_a2[:], AF.Relu, bias=0.0, scale=1.0)
                p2 = sb_pool.tile([16, nco], F32, tag="p2")  # 4*prelu(a2)
                nc.vector.scalar_tensor_tensor(out=p2[:], in0=relu2[:], scalar=3.0,
                                               in1=ps_a2[:], op0=OP.mult, op1=OP.add)
                for j in range(TG):
                    t = c * TG + j
                    nc.tensor.matmul(ps_lg[:, t:t + 1], lhsT=p2[:, j * P:(j + 1) * P],
                                     rhs=w3[:], start=True, stop=True)

            h_prev = zeros_h
            hT_stat = zeros_hT[:]
            xts = {}
            for t in range(T):
                g = t // TG
                if t % TG == 0:
                    xts[g] = _emit_gather_group(nc, g, seqidx_t, sub_table, ident,
                                                gpool, xpsum_pool, xt_pool)
                j = t % TG
                xT_stat = xts[g][:, j * P:(j + 1) * P]
                h_prev = _emit_gru_step(
                    nc, t, hT_stat, xT_stat, h_prev, gru_U, gru_Wb,
                    score=None, nscore=None, mask_bias=mask_bias[:, t:t + 1],
                    ps_pool=ps_pool, ps_ht_pool=ps_ht_pool,
                    h_pool=h_pool, zr_pool=zr_pool, tmp_pool=tmp_pool,
                    seq_hT_dst=seq_hT[0:D, t * P:(t + 1) * P], ident=ident,
                    offload_pool=nc.gpsimd if t % 2 == 0 else nc.vector)
                hT_stat = seq_hT[0:D, t * P:(t + 1) * P]
                if t % TG == TG - 1:
                    emit_att_chunk(t // TG)

            # masked softmax over T (free dim)
            lg = sb_pool.tile([P, T], F32, tag="lg")
            nc.vector.tensor_tensor(out=lg[:], in0=ps_lg[:], in1=mask_neg[:], op=OP.add)
            mrow = sb_pool.tile([P, 1], F32, tag="mrow")
            nc.vector.reduce_max(mrow[:], lg[:], axis=mybir.AxisListType.X)
            negm = sb_pool.tile([P, 1], F32, tag="negm")
            nc.vector.tensor_scalar(out=negm[:], in0=mrow[:], scalar1=-1.0,
                                    scalar2=None, op0=OP.mult)
            ex = sb_pool.tile([P, T], F32, tag="ex")
            nc.scalar.activation(ex[:], lg[:], AF.Exp, bias=negm[:, 0:1], scale=1.0)
            srow = sb_pool.tile([P, 1], F32, tag="srow")
            nc.vector.reduce_sum(srow[:], ex[:], axis=mybir.AxisListType.X)
            rrow = sb_pool.tile([P, 1], F32, tag="rrow")
            nc.vector.reciprocal(rrow[:], srow[:])
            nc.vector.tensor_scalar(out=scores[:], in0=ex[:], scalar1=rrow[:, 0:1],
                                    scalar2=None, op0=OP.mult)
            nc.vector.tensor_scalar(out=nscores[:], in0=scores[:], scalar1=-1.0,
                                    scalar2=None, op0=OP.mult)

        # ---------------- AUGRU phase ----------------
        with ExitStack() as ph:
            ps_pool = ph.enter_context(tc.tile_pool(name="aps", bufs=2, space="PSUM"))
            ps_ht_pool = ph.enter_context(tc.tile_pool(name="aps_ht", bufs=2, space="PSUM"))
            h_pool = ph.enter_context(tc.tile_pool(name="ah", bufs=4))
            zr_pool = ph.enter_context(tc.tile_pool(name="azr", bufs=2))
            tmp_pool = ph.enter_context(tc.tile_pool(name="atmp", bufs=4))
            hT_pool = ph.enter_context(tc.tile_pool(name="ahT", bufs=2))

            h_prev = zeros_h
            hT_stat = zeros_hT[:]
            for t in range(T):
                if t < T - 1:
                    hT_new = hT_pool.tile([D, P], F32, tag="ahT")
                    dst = hT_new[:]
                else:
                    dst = None  # final h^T is never read
                h_prev = _emit_gru_step(
                    nc, t, hT_stat, seq_hT[:, t * P:(t + 1) * P], h_prev, aug_U, aug_Wb,
                    score=scores[:, t:t + 1], nscore=nscores[:, t:t + 1], mask_bias=None,
                    ps_pool=ps_pool, ps_ht_pool=ps_ht_pool,
                    h_pool=h_pool, zr_pool=zr_pool, tmp_pool=tmp_pool,
                    seq_hT_dst=dst, ident=ident,
                    offload_pool=nc.gpsimd)
                if t < T - 1:
                    hT_stat = hT_new[:]

            # output: [final_state | target_embed | other_info]
            nc.sync.dma_start(out_main[:, 0:D], h_prev[:])
            nc.sync.dma_start(out_main[:, D:2 * D], tgt_emb[:])
            nc.sync.dma_start(out_main[:, 2 * D:164], other_t[:])

    nc.compile()
    return nc


# ----------------------------------------------------------------------------
# Head kernel: FFN + Dice (full-batch stats) -> [1024, 1], replicated per core
# ----------------------------------------------------------------------------

B = P * NCORES  # 1024
NT = NCORES     # row tiles of 128


def _emit_dice(nc, nfeat, pre_tiles, sq_tiles, stat_pool, sb, bc_pool, eps_ap):
    """Dice over the full batch: returns list of [128, nfeat] output tiles."""
    ps_stats = stat_pool.tile([1, 2 * nfeat], F32, tag="stats")
    ones_col = sb.tile([P, 1], F32, tag="ones_col")
    nc.gpsimd.memset(ones_col[:], 1.0)
    for r in range(NT):
        nc.tensor.matmul(ps_stats[:, 0:nfeat], lhsT=ones_col[:], rhs=pre_tiles[r][:],
                         start=(r == 0), stop=(r == NT - 1))
    for r in range(NT):
        nc.tensor.matmul(ps_stats[:, nfeat:2 * nfeat], lhsT=ones_col[:], rhs=sq_tiles[r][:],
                         start=(r == 0), stop=(r == NT - 1))
    stats = sb.tile([1, 2 * nfeat], F32, tag="stats_s")
    nc.vector.tensor_scalar(out=stats[:], in0=ps_stats[:], scalar1=1.0 / B,
                            scalar2=None, op0=OP.mult)
    msq = sb.tile([1, nfeat], F32, tag="msq")
    nc.vector.tensor_tensor(out=msq[:], in0=stats[:, 0:nfeat], in1=stats[:, 0:nfeat], op=OP.mult)
    var = sb.tile([1, nfeat], F32, tag="var")
    nc.vector.tensor_tensor(out=var[:], in0=stats[:, nfeat:2 * nfeat], in1=msq[:], op=OP.subtract)
    lnv = sb.tile([1, nfeat], F32, tag="lnv")
    nc.scalar.activation(lnv[:], var[:], AF.Ln, bias=eps_ap, scale=1.0)
    sinv = sb.tile([1, nfeat], F32, tag="sinv")
    nc.scalar.activation(sinv[:], lnv[:], AF.Exp, bias=0.0, scale=-0.5)  # rsqrt(var+eps)
    mean_bc = bc_pool.tile([P, nfeat], F32, tag="mean_bc")
    nc.gpsimd.partition_broadcast(mean_bc[:], stats[0:1, 0:nfeat])
    sinv_bc = bc_pool.tile([P, nfeat], F32, tag="sinv_bc")
    nc.gpsimd.partition_broadcast(sinv_bc[:], sinv[0:1, :])
    outs = []
    for r in range(NT):
        u = sb.tile([P, nfeat], F32, tag="u")
        nc.vector.tensor_tensor(out=u[:], in0=pre_tiles[r][:], in1=mean_bc[:], op=OP.subtract)
        un = sb.tile([P, nfeat], F32, tag="un")
        nc.vector.tensor_tensor(out=un[:], in0=u[:], in1=sinv_bc[:], op=OP.mult)
        e = sb.tile([P, nfeat], F32, tag="e")
        nc.scalar.activation(e[:], un[:], AF.Exp, bias=0.0, scale=-1.0)
        ep = sb.tile([P, nfeat], F32, tag="ep")
        nc.vector.tensor_scalar(out=ep[:], in0=e[:], scalar1=1.0, scalar2=None, op0=OP.add)
        pp = sb.tile([P, nfeat], F32, tag="pp")
        nc.vector.reciprocal(pp[:], ep[:])
        o = sb.tile([P, nfeat], F32, tag=f"do{r}")
        nc.vector.tensor_tensor(out=o[:], in0=pp[:], in1=pre_tiles[r][:], op=OP.mult)
        outs.append(o)
    return outs


def _emit_ffn_layer(nc, in_lhsT_slices, rhs_list, nfeat, ps_pool, sb):
    """One FFN layer: per row tile, psum = sum_i lhsT_i.T @ rhs_i. Returns
    (pre_tiles, sq_tiles) evacuated to SBUF."""
    pre_tiles, sq_tiles = [], []
    for r in range(NT):
        ps = ps_pool.tile([P, nfeat], F32, tag="ps_ffn")
        n = len(rhs_list)
        for i, (lhsT_fn, rhs) in enumerate(zip(in_lhsT_slices, rhs_list)):
            nc.tensor.matmul(ps[:], lhsT=lhsT_fn(r), rhs=rhs,
                             start=(i == 0), stop=(i == n - 1))
        pre = sb.tile([P, nfeat], F32, tag=f"pre{r}")
        nc.vector.tensor_copy(pre[:], ps[:])
        sq = sb.tile([P, nfeat], F32, tag=f"sq{r}")
        nc.scalar.activation(sq[:], ps[:], AF.Square, bias=0.0, scale=1.0)
        pre_tiles.append(pre)
        sq_tiles.append(sq)
    return pre_tiles, sq_tiles


def _transpose_tiles(nc, tiles, nfeat, ident, tp_pool, sb, tag):
    """Transpose [128, nfeat] row tiles into nfeat/128... -> list of [128, B]
    tensors (one per 128-wide feature block)."""
    nblk = (nfeat + P - 1) // P
    outs = [sb.tile([min(P, nfeat), B], F32, tag=f"{tag}{i}", name=f"{tag}{i}")
            for i in range(nblk)]
    for r in range(NT):
        for i in range(nblk):
            w = min(P, nfeat - i * P)
            ps = tp_pool.tile([P, P], F32, tag="tp")
            nc.tensor.transpose(out=ps[0:w, 0:P], in_=tiles[r][:, i * P:i * P + w],
                                identity=ident[:])
            nc.scalar.activation(outs[i][:, r * P:(r + 1) * P], ps[0:w, 0:P],
                                 AF.Copy, bias=0.0, scale=1.0)
    return outs


def build_head():
    nc = bacc.Bacc("TRN2", target_bir_lowering=False, debug=False)

    xT_a = nc.dram_tensor("xT_a", [P, B], F32, kind="ExternalInput")      # ffn_inT[0:128]
    xT_b = nc.dram_tensor("xT_b", [37, B], F32, kind="ExternalInput")     # [ffn_inT[128:164]; ones]
    W1a_d = nc.dram_tensor("W1a", [P, 256], F32, kind="ExternalInput")
    W1b_d = nc.dram_tensor("W1b", [37, 256], F32, kind="ExternalInput")   # [W1[128:164]; b1]
    W2_d = nc.dram_tensor("W2", [256, 128], F32, kind="ExternalInput")
    b2_d = nc.dram_tensor("b2", [1, 128], F32, kind="ExternalInput")
    W3_d = nc.dram_tensor("W3", [128, 64], F32, kind="ExternalInput")
    b3_d = nc.dram_tensor("b3", [1, 64], F32, kind="ExternalInput")
    oW_d = nc.dram_tensor("oW", [64, 1], F32, kind="ExternalInput")
    ob_d = nc.dram_tensor("ob", [1, 1], F32, kind="ExternalInput")
    out_head = nc.dram_tensor("out_head", [B, 1], F32, kind="ExternalOutput")

    with tile.TileContext(nc) as tc, ExitStack() as top:
        const = top.enter_context(tc.tile_pool(name="const", bufs=1))
        sb = top.enter_context(tc.tile_pool(name="sb", bufs=3))
        keep = top.enter_context(tc.tile_pool(name="keep", bufs=1))
        ps_pool = top.enter_context(tc.tile_pool(name="ps", bufs=2, space="PSUM"))
        tp_pool = top.enter_context(tc.tile_pool(name="tp", bufs=2, space="PSUM"))
        stat_pool = top.enter_context(tc.tile_pool(name="pstat", bufs=2, space="PSUM"))
        bc_pool = top.enter_context(tc.tile_pool(name="bc", bufs=2))

        ident = const.tile([P, P], F32)
        make_identity(nc, ident[:])
        xa = const.tile([P, B], F32); nc.sync.dma_start(xa[:], xT_a[:])
        xb = const.tile([37, B], F32); nc.sync.dma_start(xb[:], xT_b[:])
        W1a = const.tile([P, 256], F32); nc.sync.dma_start(W1a[:], W1a_d[:])
        W1b = const.tile([37, 256], F32); nc.sync.dma_start(W1b[:], W1b_d[:])
        W2a = const.tile([P, 128], F32); nc.sync.dma_start(W2a[:], W2_d[0:P, :])
        W2b = const.tile([P, 128], F32); nc.sync.dma_start(W2b[:], W2_d[P:256, :])
        b2 = const.tile([1, 128], F32); nc.sync.dma_start(b2[:], b2_d[:])
        W3 = const.tile([P, 64], F32); nc.sync.dma_start(W3[:], W3_d[:])
        b3 = const.tile([1, 64], F32); nc.sync.dma_start(b3[:], b3_d[:])
        oW = const.tile([64, 1], F32); nc.sync.dma_start(oW[:], oW_d[:])
        ob = const.tile([1, 1], F32); nc.sync.dma_start(ob[:], ob_d[:])
        ones_row = const.tile([1, P], F32); nc.gpsimd.memset(ones_row[:], 1.0)
        eps_t = const.tile([1, 1], F32); nc.gpsimd.memset(eps_t[:], 1e-9)

        keep1 = top.enter_context(tc.tile_pool(name="keep1", bufs=1))

        # L1: pre1 = x @ W1 + b1  (bias folded in xb ones row / W1b)
        pre1, sq1 = _emit_ffn_layer(
            nc,
            [lambda r: xa[:, r * P:(r + 1) * P], lambda r: xb[:, r * P:(r + 1) * P]],
            [W1a[:], W1b[:]], 256, ps_pool, keep1)
        d1 = _emit_dice(nc, 256, pre1, sq1, stat_pool, sb, bc_pool, eps_t[0:1, 0:1])

        # L2: transpose dice1 -> [256, B] as two blocks, then @ W2 + b2
        t1 = _transpose_tiles(nc, d1, 256, ident, tp_pool, keep, "t1_")
        pre2, sq2 = _emit_ffn_layer(
            nc,
            [lambda r: t1[0][:, r * P:(r + 1) * P], lambda r: t1[1][:, r * P:(r + 1) * P],
             lambda r: ones_row[:]],
            [W2a[:], W2b[:], b2[:]], 128, ps_pool, keep1)
        d2 = _emit_dice(nc, 128, pre2, sq2, stat_pool, sb, bc_pool, eps_t[0:1, 0:1])

        # L3
        t2 = _transpose_tiles(nc, d2, 128, ident, tp_pool, keep, "t2_")
        pre3, sq3 = _emit_ffn_layer(
            nc, [lambda r: t2[0][:, r * P:(r + 1) * P], lambda r: ones_row[:]],
            [W3[:], b3[:]], 64, ps_pool, keep1)
        d3 = _emit_dice(nc, 64, pre3, sq3, stat_pool, sb, bc_pool, eps_t[0:1, 0:1])

        # out = sigmoid(d3 @ oW + ob)
        t3 = _transpose_tiles(nc, d3, 64, ident, tp_pool, keep, "t3_")
        for r in range(NT):
            ps = ps_pool.tile([P, 1], F32, tag="ps_o")
            nc.tensor.matmul(ps[:], lhsT=t3[0][:, r * P:(r + 1) * P], rhs=oW[:],
                             start=True, stop=False)
            nc.tensor.matmul(ps[:], lhsT=ones_row[:], rhs=ob[:], start=False, stop=True)
            e = sb.tile([P, 1], F32, tag="oe")
            nc.scalar.activation(e[:], ps[:], AF.Exp, bias=0.0, scale=-1.0)
            ep = sb.tile([P, 1], F32, tag="oep")
            nc.vector.tensor_scalar(out=ep[:], in0=e[:], scalar1=1.0, scalar2=None, op0=OP.add)
            pp = sb.tile([P, 1], F32, tag="opp")
            nc.vector.reciprocal(pp[:], ep[:])
            nc.sync.dma_start(out_head[r * P:(r + 1) * P, :], pp[:])

    nc.compile()
    return nc


# ----------------------------------------------------------------------------
# Host driver
# ----------------------------------------------------------------------------

_MAIN_NC = None
_HEAD_NC = None
LAST_RESULTS = {}  # test harness introspection: raw BassKernelResults per launch


def pack_shared(gru_W, gru_U, gru_b, aug_W, aug_U, aug_b,
                att_W1, att_b1, att_W2, att_b2, att_W3):
    f32 = lambda x: np.asarray(x, dtype=np.float32)
    W1 = f32(att_W1)

    def uext(U):
        U = f32(U)
        return np.hstack([U[:, 0:128], U[:, 128:192], np.zeros((64, 64), np.float32)])

    def wext(W, b):
        Wb = np.vstack([f32(W), f32(b)[None, :]])
        return np.hstack([Wb[:, 0:128], np.zeros((65, 64), np.float32), Wb[:, 128:192]])

    return {
        "gru_U": uext(gru_U),
        "gru_Wb": wext(gru_W, gru_b),
        "aug_U": uext(aug_U),
        "aug_Wb": wext(aug_W, aug_b),
        "w1q": np.vstack([W1[0:64] + W1[128:192], f32(att_b1)[None, :]]),
        "w1k": np.ascontiguousarray(W1[64:128] - W1[128:192]),
        "w1qk": np.ascontiguousarray(W1[192:256]),
        "w2": np.vstack([f32(att_W2) * 0.25, f32(att_b2)[None, :]]),
        "w3": f32(att_W3) * 0.25,
    }


def _get_ncs():
    global _MAIN_NC, _HEAD_NC
    if _MAIN_NC is None:
        _MAIN_NC = build_main()
    if _HEAD_NC is None:
        _HEAD_NC = build_head()
    return _MAIN_NC, _HEAD_NC


def _prep_core(c, dense_inputs, sparse_inputs, seq_inputs, item_inputs, hist_len,
               item_table, other_tables):
    rows = slice(c * P, (c + 1) * P)
    seq = np.asarray(seq_inputs[rows, :, 0])
    tgt = np.asarray(item_inputs[rows, 0])
    all_idx = np.concatenate([seq.ravel(), tgt])
    uniq, inv = np.unique(all_idx, return_inverse=True)
    sub = np.zeros((NSUB, D), dtype=np.float32)
    sub[: uniq.shape[0]] = item_table[uniq]
    seq_remap = inv[: P * T].reshape(P, T).astype(np.int32)
    tgt_remap = inv[P * T:].reshape(P, 1).astype(np.int32)
    oth = np.concatenate(
        [np.asarray(dense_inputs[rows], dtype=np.float32)]
        + [np.asarray(other_tables[i][np.asarray(sparse_inputs[rows, i])],
                      dtype=np.float32) for i in range(2)], axis=1)
    return {
        "sub_table": sub,
        "seq_idx": np.ascontiguousarray(seq_remap),
        "tgt_idx": np.ascontiguousarray(tgt_remap),
        "hist": np.asarray(hist_len[rows], dtype=np.int32).reshape(P, 1),
        "other": np.ascontiguousarray(oth, dtype=np.float32),
    }


def kernel(dense_inputs, sparse_inputs, seq_inputs, item_inputs, hist_len,
           item_table, other_tables, gru_W, gru_U, gru_b, aug_W, aug_U, aug_b,
           att_W1, att_b1, att_W2, att_b2, att_W3, att_b3,
           ffn_W1, ffn_b1, ffn_W2, ffn_b2, ffn_W3, ffn_b3, out_W, out_b):
    main_nc, head_nc = _get_ncs()

    item_table = np.asarray(item_table, dtype=np.float32)
    f32 = lambda x: np.asarray(x, dtype=np.float32)
    shared = pack_shared(gru_W, gru_U, gru_b, aug_W, aug_U, aug_b,
                         att_W1, att_b1, att_W2, att_b2, att_W3)
    in_maps = []
    for c in range(NCORES):
        m = _prep_core(c, dense_inputs, sparse_inputs, seq_inputs, item_inputs,
                       hist_len, item_table, other_tables)
        m.update(shared)
        in_maps.append(m)

    import time as _time
    t0 = _time.perf_counter()
    bres = run_bass_kernel_spmd(main_nc, in_maps, list(range(NCORES)))
    LAST_RESULTS["main"] = bres
    LAST_RESULTS["main_wall_s"] = _time.perf_counter() - t0
    res = bres.results
    ffn_in = np.concatenate([res[c]["out_main"] for c in range(NCORES)], axis=0)

    ffn_inT = np.ascontiguousarray(ffn_in.T)  # [164, 1024]
    head_map = {
        "xT_a": ffn_inT[0:128],
        "xT_b": np.vstack([ffn_inT[128:164], np.ones((1, B), np.float32)]),
        "W1a": f32(ffn_W1)[0:128],
        "W1b": np.vstack([f32(ffn_W1)[128:164], f32(ffn_b1)[None, :]]),
        "W2": f32(ffn_W2), "b2": f32(ffn_b2)[None, :],
        "W3": f32(ffn_W3), "b3": f32(ffn_b3)[None, :],
        "oW": f32(out_W), "ob": f32(out_b)[None, :],
    }
    t0 = _time.perf_counter()
    hbres = run_bass_kernel_spmd(head_nc, [head_map] * NCORES, list(range(NCORES)))
    LAST_RESULTS["head"] = hbres
    LAST_RESULTS["head_wall_s"] = _time.perf_counter() - t0
    return np.asarray(hbres.results[0]["out_head"], dtype=np.float32)
